# revision 9
# baseline (speedup 1.0000x reference)
"""Trainium2 Bass kernel for banded (local-causal) multi-head self-attention.

Problem (hardcoded shapes): x [4, 2048, 1024], W_attn [1024, 3072],
b_attn [3072], W_proj [1024, 1024], b_proj [1024]; 16 heads, head dim 64,
local causal window 256.

Sharding over 8 NeuronCores: data-parallel over the 4 batches x
tensor-parallel over 2 head-groups (8 heads each). Each core computes a
partial projection output [2048, 1024]; the host sums the two head-group
partials per batch and adds b_proj.

Per-core device program (all loops fully unrolled under Tile):
  phase 1: qkv projections.  q^T/k^T produced feature-major [feat, T] so the
           attention contraction (over head dim) has feat on partitions; v is
           produced token-major [T, feat] interleaved with a ones column per
           head ([1 | v_h] stride 65) so the PV matmul also emits the softmax
           denominator.
  phase 2: banded attention per head over T-strips.  For key-block i the
           valid queries are tq in [128i, 128i+384), so one [64x128]^T @
           [64, w] matmul gives the S^T strip, one Exp activation (scale =
           1/sqrt(64)) and one band-mask multiply give the E^T strip, and
           three [128, 65]^T @ [128, 128] matmuls accumulate A^T (rows 1..64)
           and the denominator s (row 0) in PSUM.  s is reciprocal'd,
           broadcast across partitions with a K=1 ones matmul, and multiplied
           into A^T during the PSUM->SBUF drain.
  phase 3: output projection from a^T [512, T] tiles (float32r matmuls).

Matmul dtypes: bf16 for qkv + attention (inputs pre-cast on host), float32r
for the output projection.  All accumulation is fp32 in PSUM.
"""

import os
import numpy as np
import ml_dtypes

import concourse.bass as bass
import concourse.bacc as bacc
import concourse.mybir as mybir
import concourse.tile as tile
from concourse.bass_utils import run_bass_kernel_spmd

B, T, C = 4, 2048, 1024
H, D, CTX = 16, 64, 256
HG = 8                 # heads per core
FG = HG * D            # 512 features per group
P = 128
NT = T // P            # 16 token blocks
KC = C // P            # 8 contraction tiles of C
W3 = 3 * P             # strip width 384

BF16 = mybir.dt.bfloat16
F32 = mybir.dt.float32
F32R = mybir.dt.float32r

# set by the last kernel() call; test harness reads exec_time_ns from here
LAST_RESULTS = None

_BUILD_CACHE = {}


def _build_nc(qk_bias: bool, v_bias: bool) -> bass.Bass:
    nc = bacc.Bacc()

    xt_d = nc.declare_dram_parameter("xt", [C, T], BF16, isOutput=False)
    wqk_d = nc.declare_dram_parameter("wqk", [C, 2 * FG], BF16, isOutput=False)
    wv_d = nc.declare_dram_parameter("wv", [C, FG], BF16, isOutput=False)
    wp_d = nc.declare_dram_parameter("wp", [FG, C], F32R, isOutput=False)
    mstrip_d = nc.declare_dram_parameter("mstrip", [P, W3], BF16, isOutput=False)
    if qk_bias:
        bqk_d = nc.declare_dram_parameter("bqk", [8, P], F32, isOutput=False)
    if v_bias:
        bv_d = nc.declare_dram_parameter("bv", [P, HG * (D + 1)], BF16, isOutput=False)
    y_d = nc.declare_dram_parameter("y", [T, C], F32, isOutput=True)

    with tile.TileContext(nc) as tc:
        with tc.tile_pool(name="const", bufs=1) as const, \
             tc.tile_pool(name="stage", bufs=8) as stage_p, \
             tc.tile_pool(name="epool", bufs=4) as e_pool, \
             tc.tile_pool(name="ypool", bufs=3) as y_pool:

            # ---- resident SBUF tiles -------------------------------------
            xt = [const.tile([P, T], BF16, tag=f"xt{k}", name=f"xt{k}") for k in range(KC)]
            wqk = [const.tile([P, 2 * FG], BF16, tag=f"wqk{k}", name=f"wqk{k}") for k in range(KC)]
            wv = [const.tile([P, FG], BF16, tag=f"wv{k}", name=f"wv{k}") for k in range(KC)]
            wp = [const.tile([P, C], F32R, tag=f"wp{k}", name=f"wp{k}") for k in range(FG // P)]
            qkT = [const.tile([P, T], BF16, tag=f"qkT{f}", name=f"qkT{f}") for f in range(8)]
            vag = [const.tile([P, HG * (D + 1)], BF16, tag=f"vag{t}", name=f"vag{t}") for t in range(NT)]
            aT = [const.tile([P, T], F32R, tag=f"aT{k}", name=f"aT{k}") for k in range(FG // P)]
            mask_t = const.tile([P, W3], BF16, tag="mask", name="mask")
            ones_t = const.tile([P, D], F32, tag="ones", name="ones")

            nc.any.memset(ones_t[:], 1.0)
            nc.sync.dma_start(mask_t[:], mstrip_d[:])
            xt_r = xt_d.rearrange("(a p) t -> a p t", p=P)
            wqk_r = wqk_d.rearrange("(a p) f -> a p f", p=P)
            wv_r = wv_d.rearrange("(a p) f -> a p f", p=P)
            wp_r = wp_d.rearrange("(a p) f -> a p f", p=P)
            for k in range(KC):
                nc.sync.dma_start(xt[k][:], xt_r[k])
                nc.sync.dma_start(wqk[k][:], wqk_r[k])
                nc.sync.dma_start(wv[k][:], wv_r[k])
            for k in range(FG // P):
                nc.sync.dma_start(wp[k][:], wp_r[k])
            if qk_bias:
                bqk_t = const.tile([P, 8], F32, tag="bqk", name="bqk")
                nc.sync.dma_start(bqk_t[:], bqk_d.rearrange("a p -> p a"))
            if v_bias:
                bv_t = const.tile([P, HG * (D + 1)], BF16, tag="bv", name="bv")
                nc.sync.dma_start(bv_t[:], bv_d[:])

            # ---- phase 1: qkv projections --------------------------------
            with tc.tile_pool(name="ps_qkv", bufs=3, space="PSUM") as ps_qkv:
                # q^T / k^T: feature-major [128 feat, T]
                for ft in range(8):
                    for nt in range(T // 512):
                        ps = ps_qkv.tile([P, 512], F32, tag="qkv", name="qkv")
                        for k in range(KC):
                            nc.tensor.matmul(
                                ps[:],
                                lhsT=wqk[k][:, ft * P:(ft + 1) * P],
                                rhs=xt[k][:, nt * 512:(nt + 1) * 512],
                                start=(k == 0),
                                stop=(k == KC - 1),
                            )
                        dst = qkT[ft][:, nt * 512:(nt + 1) * 512]
                        if qk_bias:
                            nc.scalar.activation(
                                dst, ps[:],
                                mybir.ActivationFunctionType.Copy,
                                bias=bqk_t[:, ft:ft + 1],
                            )
                        else:
                            nc.any.tensor_copy(dst, ps[:])
                # v: token-major [128 tok, FG], scattered into [1|v_h] slots
                for t in range(NT):
                    ps = ps_qkv.tile([P, 512], F32, tag="qkv", name="qkv")
                    for k in range(KC):
                        nc.tensor.matmul(
                            ps[:],
                            lhsT=xt[k][:, t * P:(t + 1) * P],
                            rhs=wv[k][:],
                            start=(k == 0),
                            stop=(k == KC - 1),
                        )
                    vv = vag[t].rearrange("p (h c) -> p h c", c=D + 1)
                    nc.any.memset(vv[:, :, D:D + 1], 1.0)
                    nc.any.tensor_copy(
                        vv[:, :, 0:D],
                        ps.rearrange("p (h c) -> p h c", c=D),
                    )
                    if v_bias:
                        nc.vector.tensor_add(vag[t][:], vag[t][:], bv_t[:])

            # ---- phase 2: banded attention -------------------------------
            with tc.tile_pool(name="ps_s", bufs=2, space="PSUM") as ps_s, \
                 tc.tile_pool(name="ps_a", bufs=4, space="PSUM") as ps_a, \
                 tc.tile_pool(name="ps_r", bufs=2, space="PSUM") as ps_r:
                for h in range(HG):
                    hp, ho = h // 2, (h % 2) * D
                    q_ap = qkT[hp]
                    k_ap = qkT[4 + hp]
                    psA = {}
                    for i in range(NT):
                        w = min(W3, (NT - i) * P)
                        ps = ps_s.tile([P, W3], F32, tag="s", name="s")
                        nc.tensor.matmul(
                            ps[:, :w],
                            lhsT=k_ap[ho:ho + D, i * P:(i + 1) * P],
                            rhs=q_ap[ho:ho + D, i * P:i * P + w],
                            start=True, stop=True,
                        )
                        e_t = e_pool.tile([P, W3], BF16, tag="e", name="e")
                        nc.scalar.activation(
                            e_t[:, :w], ps[:, :w],
                            mybir.ActivationFunctionType.Exp,
                            scale=0.125,
                        )
                        nc.vector.tensor_mul(e_t[:, :w], e_t[:, :w], mask_t[:, :w])
                        for dj in range(3):
                            j = i + dj
                            if j >= NT:
                                continue
                            first = (i == max(0, j - 2))
                            last = (i == j)
                            if first:
                                psA[j] = ps_a.tile([D + 1, P], F32, tag="psA", name="psA")
                            nc.tensor.matmul(
                                psA[j][:],
                                lhsT=vag[i][:, h * (D + 1):(h + 1) * (D + 1)],
                                rhs=e_t[:, dj * P:(dj + 1) * P],
                                start=first, stop=last,
                            )
                            if last:
                                pa = psA.pop(j)
                                st = stage_p.tile([D + 1, P], F32, tag="st", name="st")
                                nc.any.tensor_copy(st[:], pa[:])
                                rst = stage_p.tile([D + 1, P], F32, tag="rst", name="rst")
                                nc.vector.reciprocal(rst[D:D + 1, :], st[D:D + 1, :])
                                rb = ps_r.tile([D, P], F32, tag="rb", name="rb")
                                nc.tensor.matmul(
                                    rb[:], lhsT=ones_t[D:D + 1, :], rhs=rst[D:D + 1, :],
                                    start=True, stop=True,
                                )
                                nc.vector.tensor_mul(
                                    aT[h // 2][ho:ho + D, j * P:(j + 1) * P],
                                    st[0:D, :],
                                    rb[:],
                                )
                    assert not psA

            # ---- phase 3: output projection ------------------------------
            with tc.tile_pool(name="ps_y", bufs=3, space="PSUM") as ps_y:
                for j in range(NT):
                    yt = y_pool.tile([P, C], F32, tag="y", name="y")
                    for n in range(C // 512):
                        ps = ps_y.tile([P, 512], F32, tag="py", name="py")
                        for k2 in range(FG // P):
                            nc.tensor.matmul(
                                ps[:],
                                lhsT=aT[k2][:, j * P:(j + 1) * P],
                                rhs=wp[k2][:, n * 512:(n + 1) * 512],
                                start=(k2 == 0),
                                stop=(k2 == FG // P - 1),
                            )
                        nc.any.tensor_copy(yt[:, n * 512:(n + 1) * 512], ps[:])
                    nc.sync.dma_start(y_d[j * P:(j + 1) * P, :], yt[:])

    nc.finalize()
    return nc


def _band_mask_strip() -> np.ndarray:
    tk = np.arange(P)[:, None]
    tq = np.arange(W3)[None, :]
    return (((tq - tk) >= 0) & ((tq - tk) < CTX)).astype(ml_dtypes.bfloat16)


def kernel(x, W_attn, b_attn, W_proj, b_proj):
    global LAST_RESULTS
    x = np.asarray(x, dtype=np.float32)
    W_attn = np.asarray(W_attn, dtype=np.float32)
    b_attn = np.asarray(b_attn, dtype=np.float32)
    W_proj = np.asarray(W_proj, dtype=np.float32)
    b_proj = np.asarray(b_proj, dtype=np.float32)

    qk_bias = bool(np.any(b_attn[:2 * C]))
    v_bias = bool(np.any(b_attn[2 * C:]))

    key = (qk_bias, v_bias)
    if key not in _BUILD_CACHE:
        _BUILD_CACHE[key] = _build_nc(qk_bias, v_bias)
    nc = _BUILD_CACHE[key]

    mstrip = _band_mask_strip()
    in_maps = []
    for c in range(8):
        b, g = c // 2, c % 2
        fsl = slice(FG * g, FG * (g + 1))
        im = {
            "xt": np.ascontiguousarray(x[b].T).astype(ml_dtypes.bfloat16),
            "wqk": np.ascontiguousarray(
                np.concatenate(
                    [W_attn[:, fsl], W_attn[:, C + FG * g:C + FG * (g + 1)]],
                    axis=1,
                )
            ).astype(ml_dtypes.bfloat16),
            "wv": np.ascontiguousarray(
                W_attn[:, 2 * C + FG * g:2 * C + FG * (g + 1)]
            ).astype(ml_dtypes.bfloat16),
            "wp": np.ascontiguousarray(W_proj[fsl, :]),
            "mstrip": mstrip,
        }
        if qk_bias:
            bq = b_attn[fsl]
            bk = b_attn[C + FG * g:C + FG * (g + 1)]
            im["bqk"] = np.concatenate([bq, bk]).reshape(8, P).astype(np.float32)
        if v_bias:
            bv = b_attn[2 * C + FG * g:2 * C + FG * (g + 1)]
            bvt = np.zeros((HG, D + 1), dtype=np.float32)
            bvt[:, 1:] = bv.reshape(HG, D)
            im["bv"] = np.broadcast_to(
                bvt.reshape(1, HG * (D + 1)), (P, HG * (D + 1))
            ).astype(ml_dtypes.bfloat16)
        in_maps.append(im)

    res = run_bass_kernel_spmd(nc, in_maps, list(range(8)))
    LAST_RESULTS = res

    out = np.empty((B, T, C), dtype=np.float32)
    for b in range(B):
        out[b] = res.results[2 * b]["y"] + res.results[2 * b + 1]["y"] + b_proj
    return out


# revision 14
# speedup vs baseline: 1.4240x; 1.4240x over previous
"""Trainium2 Bass kernel for banded (local-causal) multi-head self-attention.

Problem (hardcoded shapes): x [4, 2048, 1024], W_attn [1024, 3072],
b_attn [3072], W_proj [1024, 1024], b_proj [1024]; 16 heads, head dim 64,
local causal window 256.

Sharding over 8 NeuronCores: data-parallel over the 4 batches x
tensor-parallel over 2 head-groups (8 heads each). Each core computes a
partial projection output [2048, 1024]; the host sums the two head-group
partials per batch and adds b_proj.

Per-core device program (all loops fully unrolled under Tile):
  phase 1: qkv projections.  q^T/k^T produced feature-major [feat, T] so the
           attention contraction (over head dim) has feat on partitions; v is
           produced token-major [T, feat] interleaved with a ones column per
           head ([1 | v_h] stride 65) so the PV matmul also emits the softmax
           denominator.
  phase 2: banded attention per head over T-strips.  For key-block i the
           valid queries are tq in [128i, 128i+384), so one [64x128]^T @
           [64, w] matmul gives the S^T strip, one Exp activation (scale =
           1/sqrt(64)) and one band-mask multiply give the E^T strip, and
           three [128, 65]^T @ [128, 128] matmuls accumulate A^T (rows 1..64)
           and the denominator s (row 0) in PSUM.  s is reciprocal'd,
           broadcast across partitions with a K=1 ones matmul, and multiplied
           into A^T during the PSUM->SBUF drain.
  phase 3: output projection from a^T [512, T] tiles (float32r matmuls).

Matmul dtypes: bf16 for qkv + attention (inputs pre-cast on host), float32r
for the output projection.  All accumulation is fp32 in PSUM.
"""

import os
import numpy as np
import ml_dtypes

import concourse.bass as bass
import concourse.bacc as bacc
import concourse.mybir as mybir
import concourse.tile as tile
from concourse.bass_utils import run_bass_kernel_spmd

B, T, C = 4, 2048, 1024
H, D, CTX = 16, 64, 256
HG = 8                 # heads per core
FG = HG * D            # 512 features per group
P = 128
NT = T // P            # 16 token blocks
KC = C // P            # 8 contraction tiles of C
W3 = 3 * P             # strip width 384

BF16 = mybir.dt.bfloat16
F32 = mybir.dt.float32
F32R = mybir.dt.float32r

# set by the last kernel() call; test harness reads exec_time_ns from here
LAST_RESULTS = None

_BUILD_CACHE = {}


def _build_nc(qk_bias: bool, v_bias: bool) -> bass.Bass:
    nc = bacc.Bacc()

    xt_d = nc.declare_dram_parameter("xt", [C, T], BF16, isOutput=False)
    wqk_d = nc.declare_dram_parameter("wqk", [C, 2 * FG], BF16, isOutput=False)
    wv_d = nc.declare_dram_parameter("wv", [C, FG], BF16, isOutput=False)
    wp_d = nc.declare_dram_parameter("wp", [FG, C], BF16, isOutput=False)
    mstrip_d = nc.declare_dram_parameter("mstrip", [P, W3], BF16, isOutput=False)
    if qk_bias:
        bqk_d = nc.declare_dram_parameter("bqk", [8, P], F32, isOutput=False)
    if v_bias:
        bv_d = nc.declare_dram_parameter("bv", [P, HG * (D + 1)], BF16, isOutput=False)
    y_d = nc.declare_dram_parameter("y", [T, C], F32, isOutput=True)

    with tile.TileContext(nc) as tc:
        with tc.tile_pool(name="const", bufs=1) as const, \
             tc.tile_pool(name="stage", bufs=8) as stage_p, \
             tc.tile_pool(name="epool", bufs=4) as e_pool, \
             tc.tile_pool(name="ypool", bufs=3) as y_pool:

            # ---- resident SBUF tiles -------------------------------------
            xt = [const.tile([P, T], BF16, tag=f"xt{k}", name=f"xt{k}") for k in range(KC)]
            wqk = [const.tile([P, 2 * FG], BF16, tag=f"wqk{k}", name=f"wqk{k}") for k in range(KC)]
            wv = [const.tile([P, FG], BF16, tag=f"wv{k}", name=f"wv{k}") for k in range(KC)]
            wp = [const.tile([P, C], BF16, tag=f"wp{k}", name=f"wp{k}") for k in range(FG // P)]
            qkT = [const.tile([P, T], BF16, tag=f"qkT{f}", name=f"qkT{f}") for f in range(8)]
            vag = [const.tile([P, HG * (D + 1)], BF16, tag=f"vag{t}", name=f"vag{t}") for t in range(NT)]
            a_nat = [const.tile([P, FG], BF16, tag=f"an{t}", name=f"an{t}") for t in range(NT)]
            aTb = [const.tile([P, T], BF16, tag=f"aTb{k}", name=f"aTb{k}") for k in range(FG // P)]
            mask_t = const.tile([P, W3], BF16, tag="mask", name="mask")

            nc.sync.dma_start(mask_t[:], mstrip_d[:])
            xt_r = xt_d.rearrange("(a p) t -> a p t", p=P)
            wqk_r = wqk_d.rearrange("(a p) f -> a p f", p=P)
            wv_r = wv_d.rearrange("(a p) f -> a p f", p=P)
            wp_r = wp_d.rearrange("(a p) f -> a p f", p=P)
            for k in range(KC):
                nc.sync.dma_start(xt[k][:], xt_r[k])
                nc.sync.dma_start(wqk[k][:], wqk_r[k])
                nc.sync.dma_start(wv[k][:], wv_r[k])
            for k in range(FG // P):
                nc.sync.dma_start(wp[k][:], wp_r[k])
            if qk_bias:
                bqk_t = const.tile([P, 8], F32, tag="bqk", name="bqk")
                nc.sync.dma_start(bqk_t[:], bqk_d.rearrange("a p -> p a"))
            if v_bias:
                bv_t = const.tile([P, HG * (D + 1)], BF16, tag="bv", name="bv")
                nc.sync.dma_start(bv_t[:], bv_d[:])

            # ---- phase 1: qkv projections --------------------------------
            with tc.tile_pool(name="ps_qkv", bufs=3, space="PSUM") as ps_qkv:
                # q^T / k^T: feature-major [128 feat, T]
                for ft in range(8):
                    for nt in range(T // 512):
                        ps = ps_qkv.tile([P, 512], F32, tag="qkv", name="qkv")
                        for k in range(KC):
                            nc.tensor.matmul(
                                ps[:],
                                lhsT=wqk[k][:, ft * P:(ft + 1) * P],
                                rhs=xt[k][:, nt * 512:(nt + 1) * 512],
                                start=(k == 0),
                                stop=(k == KC - 1),
                            )
                        dst = qkT[ft][:, nt * 512:(nt + 1) * 512]
                        if qk_bias:
                            nc.scalar.activation(
                                dst, ps[:],
                                mybir.ActivationFunctionType.Copy,
                                bias=bqk_t[:, ft:ft + 1],
                            )
                        else:
                            nc.any.tensor_copy(dst, ps[:])
                # v: token-major [128 tok, FG], scattered into [1|v_h] slots
                for t in range(NT):
                    ps = ps_qkv.tile([P, 512], F32, tag="qkv", name="qkv")
                    for k in range(KC):
                        nc.tensor.matmul(
                            ps[:],
                            lhsT=xt[k][:, t * P:(t + 1) * P],
                            rhs=wv[k][:],
                            start=(k == 0),
                            stop=(k == KC - 1),
                        )
                    vv = vag[t].rearrange("p (h c) -> p h c", c=D + 1)
                    nc.any.memset(vv[:, :, D:D + 1], 1.0)
                    nc.any.tensor_copy(
                        vv[:, :, 0:D],
                        ps.rearrange("p (h c) -> p h c", c=D),
                    )
                    if v_bias:
                        nc.vector.tensor_add(vag[t][:], vag[t][:], bv_t[:])

            # ---- phase 2: banded attention -------------------------------
            # PV uses E^T blocks as the stationary operand so the output is
            # A natural [tq, 65] with the softmax denominator in column 64 —
            # per-partition, so normalization is reciprocal + tensor_scalar.
            with tc.tile_pool(name="ps_s", bufs=3, space="PSUM") as ps_s, \
                 tc.tile_pool(name="ps_a", bufs=4, space="PSUM") as ps_a:
                for h in range(HG):
                    hp, ho = h // 2, (h % 2) * D
                    q_ap = qkT[hp]
                    k_ap = qkT[4 + hp]
                    psA = {}
                    for i in range(NT):
                        w = min(W3, (NT - i) * P)
                        ps = ps_s.tile([P, W3], F32, tag="s", name="s")
                        nc.tensor.matmul(
                            ps[:, :w],
                            lhsT=k_ap[ho:ho + D, i * P:(i + 1) * P],
                            rhs=q_ap[ho:ho + D, i * P:i * P + w],
                            start=True, stop=True,
                        )
                        e_t = e_pool.tile([P, W3], BF16, tag="e", name="e")
                        nc.scalar.activation(
                            e_t[:, :w], ps[:, :w],
                            mybir.ActivationFunctionType.Exp,
                            scale=0.125,
                        )
                        # band mask; alternate engines to balance load
                        if h % 2 == 0:
                            nc.vector.tensor_mul(e_t[:, :w], e_t[:, :w], mask_t[:, :w])
                        else:
                            nc.gpsimd.tensor_mul(e_t[:, :w], e_t[:, :w], mask_t[:, :w])
                        for dj in range(3):
                            j = i + dj
                            if j >= NT:
                                continue
                            first = (i == max(0, j - 2))
                            last = (i == j)
                            if first:
                                psA[j] = ps_a.tile([P, D + 1], F32, tag="psA", name="psA")
                            nc.tensor.matmul(
                                psA[j][:],
                                lhsT=e_t[:, dj * P:(dj + 1) * P],
                                rhs=vag[i][:, h * (D + 1):(h + 1) * (D + 1)],
                                start=first, stop=last,
                            )
                            if last:
                                pa = psA.pop(j)
                                rs = stage_p.tile([P, 1], F32, tag="rs", name="rs")
                                nc.vector.reciprocal(rs[:], pa[:, D:D + 1])
                                nc.vector.tensor_scalar(
                                    a_nat[j][:, h * D:(h + 1) * D],
                                    pa[:, 0:D],
                                    rs[:],
                                    None,
                                    mybir.AluOpType.mult,
                                )
                    assert not psA

            # transpose a_nat [tq, c] -> aTb [c, tq] (bf16, via DMA xbar)
            for j in range(NT):
                for k2 in range(FG // P):
                    nc.sync.dma_start_transpose(
                        aTb[k2][:, j * P:(j + 1) * P],
                        a_nat[j][:, k2 * P:(k2 + 1) * P],
                    )

            # ---- phase 3: output projection ------------------------------
            with tc.tile_pool(name="ps_y", bufs=3, space="PSUM") as ps_y:
                for j in range(NT):
                    yt = y_pool.tile([P, C], F32, tag="y", name="y")
                    for n in range(C // 512):
                        ps = ps_y.tile([P, 512], F32, tag="py", name="py")
                        for k2 in range(FG // P):
                            nc.tensor.matmul(
                                ps[:],
                                lhsT=aTb[k2][:, j * P:(j + 1) * P],
                                rhs=wp[k2][:, n * 512:(n + 1) * 512],
                                start=(k2 == 0),
                                stop=(k2 == FG // P - 1),
                            )
                        nc.any.tensor_copy(yt[:, n * 512:(n + 1) * 512], ps[:])
                    nc.sync.dma_start(y_d[j * P:(j + 1) * P, :], yt[:])

    nc.finalize()
    return nc


def _band_mask_strip() -> np.ndarray:
    tk = np.arange(P)[:, None]
    tq = np.arange(W3)[None, :]
    return (((tq - tk) >= 0) & ((tq - tk) < CTX)).astype(ml_dtypes.bfloat16)


def kernel(x, W_attn, b_attn, W_proj, b_proj):
    global LAST_RESULTS
    x = np.asarray(x, dtype=np.float32)
    W_attn = np.asarray(W_attn, dtype=np.float32)
    b_attn = np.asarray(b_attn, dtype=np.float32)
    W_proj = np.asarray(W_proj, dtype=np.float32)
    b_proj = np.asarray(b_proj, dtype=np.float32)

    qk_bias = bool(np.any(b_attn[:2 * C]))
    v_bias = bool(np.any(b_attn[2 * C:]))

    key = (qk_bias, v_bias)
    if key not in _BUILD_CACHE:
        _BUILD_CACHE[key] = _build_nc(qk_bias, v_bias)
    nc = _BUILD_CACHE[key]

    mstrip = _band_mask_strip()
    in_maps = []
    for c in range(8):
        b, g = c // 2, c % 2
        fsl = slice(FG * g, FG * (g + 1))
        im = {
            "xt": np.ascontiguousarray(x[b].T).astype(ml_dtypes.bfloat16),
            "wqk": np.ascontiguousarray(
                np.concatenate(
                    [W_attn[:, fsl], W_attn[:, C + FG * g:C + FG * (g + 1)]],
                    axis=1,
                )
            ).astype(ml_dtypes.bfloat16),
            "wv": np.ascontiguousarray(
                W_attn[:, 2 * C + FG * g:2 * C + FG * (g + 1)]
            ).astype(ml_dtypes.bfloat16),
            "wp": np.ascontiguousarray(W_proj[fsl, :]).astype(ml_dtypes.bfloat16),
            "mstrip": mstrip,
        }
        if qk_bias:
            bq = b_attn[fsl]
            bk = b_attn[C + FG * g:C + FG * (g + 1)]
            im["bqk"] = np.concatenate([bq, bk]).reshape(8, P).astype(np.float32)
        if v_bias:
            bv = b_attn[2 * C + FG * g:2 * C + FG * (g + 1)]
            bvt = np.zeros((HG, D + 1), dtype=np.float32)
            bvt[:, 1:] = bv.reshape(HG, D)
            im["bv"] = np.broadcast_to(
                bvt.reshape(1, HG * (D + 1)), (P, HG * (D + 1))
            ).astype(ml_dtypes.bfloat16)
        in_maps.append(im)

    res = run_bass_kernel_spmd(nc, in_maps, list(range(8)))
    LAST_RESULTS = res

    out = np.empty((B, T, C), dtype=np.float32)
    for b in range(B):
        out[b] = res.results[2 * b]["y"] + res.results[2 * b + 1]["y"] + b_proj
    return out


# revision 15
# speedup vs baseline: 1.5033x; 1.0557x over previous
"""Trainium2 Bass kernel for banded (local-causal) multi-head self-attention.

Problem (hardcoded shapes): x [4, 2048, 1024], W_attn [1024, 3072],
b_attn [3072], W_proj [1024, 1024], b_proj [1024]; 16 heads, head dim 64,
local causal window 256.

Sharding over 8 NeuronCores: data-parallel over the 4 batches x
tensor-parallel over 2 head-groups (8 heads each). Each core computes a
partial projection output [2048, 1024]; the host sums the two head-group
partials per batch and adds b_proj.

Per-core device program (all loops fully unrolled under Tile):
  phase 1: qkv projections.  q^T/k^T produced feature-major [feat, T] so the
           attention contraction (over head dim) has feat on partitions; v is
           produced token-major [T, feat] interleaved with a ones column per
           head ([1 | v_h] stride 65) so the PV matmul also emits the softmax
           denominator.
  phase 2: banded attention per head over T-strips.  For key-block i the
           valid queries are tq in [128i, 128i+384), so one [64x128]^T @
           [64, w] matmul gives the S^T strip, one Exp activation (scale =
           1/sqrt(64)) and one band-mask multiply give the E^T strip, and
           three [128, 65]^T @ [128, 128] matmuls accumulate A^T (rows 1..64)
           and the denominator s (row 0) in PSUM.  s is reciprocal'd,
           broadcast across partitions with a K=1 ones matmul, and multiplied
           into A^T during the PSUM->SBUF drain.
  phase 3: output projection from a^T [512, T] tiles (float32r matmuls).

Matmul dtypes: bf16 for qkv + attention (inputs pre-cast on host), float32r
for the output projection.  All accumulation is fp32 in PSUM.
"""

import os
import numpy as np
import ml_dtypes

import concourse.bass as bass
import concourse.bacc as bacc
import concourse.mybir as mybir
import concourse.tile as tile
from concourse.bass_utils import run_bass_kernel_spmd

B, T, C = 4, 2048, 1024
H, D, CTX = 16, 64, 256
HG = 8                 # heads per core
FG = HG * D            # 512 features per group
P = 128
NT = T // P            # 16 token blocks
KC = C // P            # 8 contraction tiles of C
W3 = 3 * P             # strip width 384

BF16 = mybir.dt.bfloat16
F32 = mybir.dt.float32
F32R = mybir.dt.float32r

# set by the last kernel() call; test harness reads exec_time_ns from here
LAST_RESULTS = None

_BUILD_CACHE = {}


def _build_nc(qk_bias: bool, v_bias: bool) -> bass.Bass:
    nc = bacc.Bacc()

    xt_d = nc.declare_dram_parameter("xt", [C, T], BF16, isOutput=False)
    wqk_d = nc.declare_dram_parameter("wqk", [C, 2 * FG], BF16, isOutput=False)
    wv_d = nc.declare_dram_parameter("wv", [C, FG], BF16, isOutput=False)
    wp_d = nc.declare_dram_parameter("wp", [FG, C], BF16, isOutput=False)
    mstrip_d = nc.declare_dram_parameter("mstrip", [P, W3], BF16, isOutput=False)
    if qk_bias:
        bqk_d = nc.declare_dram_parameter("bqk", [8, P], F32, isOutput=False)
    if v_bias:
        bv_d = nc.declare_dram_parameter("bv", [P, HG * (D + 1)], BF16, isOutput=False)
    y_d = nc.declare_dram_parameter("y", [T, C], F32, isOutput=True)

    with tile.TileContext(nc) as tc:
        with tc.tile_pool(name="const", bufs=1) as const, \
             tc.tile_pool(name="stage", bufs=8) as stage_p, \
             tc.tile_pool(name="epool", bufs=6) as e_pool, \
             tc.tile_pool(name="ypool", bufs=3) as y_pool:

            # ---- resident SBUF tiles -------------------------------------
            xt = [const.tile([P, T], BF16, tag=f"xt{k}", name=f"xt{k}") for k in range(KC)]
            wqk = [const.tile([P, 2 * FG], BF16, tag=f"wqk{k}", name=f"wqk{k}") for k in range(KC)]
            wv = [const.tile([P, FG], BF16, tag=f"wv{k}", name=f"wv{k}") for k in range(KC)]
            wp = [const.tile([P, C], BF16, tag=f"wp{k}", name=f"wp{k}") for k in range(FG // P)]
            qkT = [const.tile([P, T], BF16, tag=f"qkT{f}", name=f"qkT{f}") for f in range(8)]
            vag = [const.tile([P, HG * (D + 1)], BF16, tag=f"vag{t}", name=f"vag{t}") for t in range(NT)]
            a_nat = [const.tile([P, FG], BF16, tag=f"an{t}", name=f"an{t}") for t in range(NT)]
            aTb = [const.tile([P, T], BF16, tag=f"aTb{k}", name=f"aTb{k}") for k in range(FG // P)]
            mask_t = const.tile([P, W3], BF16, tag="mask", name="mask")

            nc.sync.dma_start(mask_t[:], mstrip_d[:])
            xt_r = xt_d.rearrange("(a p) t -> a p t", p=P)
            wqk_r = wqk_d.rearrange("(a p) f -> a p f", p=P)
            wv_r = wv_d.rearrange("(a p) f -> a p f", p=P)
            wp_r = wp_d.rearrange("(a p) f -> a p f", p=P)
            for k in range(KC):
                nc.sync.dma_start(wqk[k][:], wqk_r[k])
                nc.sync.dma_start(xt[k][:, 0:T // 2], xt_r[k][:, 0:T // 2])
            for k in range(KC):
                nc.sync.dma_start(xt[k][:, T // 2:T], xt_r[k][:, T // 2:T])
                nc.sync.dma_start(wv[k][:], wv_r[k])
            for k in range(FG // P):
                nc.sync.dma_start(wp[k][:], wp_r[k])
            if qk_bias:
                bqk_t = const.tile([P, 8], F32, tag="bqk", name="bqk")
                nc.sync.dma_start(bqk_t[:], bqk_d.rearrange("a p -> p a"))
            if v_bias:
                bv_t = const.tile([P, HG * (D + 1)], BF16, tag="bv", name="bv")
                nc.sync.dma_start(bv_t[:], bv_d[:])

            # ---- phase 1: qkv projections --------------------------------
            with tc.tile_pool(name="ps_qkv", bufs=3, space="PSUM") as ps_qkv:
                # q^T / k^T: feature-major [128 feat, T]
                for ft in range(8):
                    for nt in range(T // 512):
                        ps = ps_qkv.tile([P, 512], F32, tag="qkv", name="qkv")
                        for k in range(KC):
                            nc.tensor.matmul(
                                ps[:],
                                lhsT=wqk[k][:, ft * P:(ft + 1) * P],
                                rhs=xt[k][:, nt * 512:(nt + 1) * 512],
                                start=(k == 0),
                                stop=(k == KC - 1),
                            )
                        dst = qkT[ft][:, nt * 512:(nt + 1) * 512]
                        if qk_bias:
                            nc.scalar.activation(
                                dst, ps[:],
                                mybir.ActivationFunctionType.Copy,
                                bias=bqk_t[:, ft:ft + 1],
                            )
                        else:
                            nc.any.tensor_copy(dst, ps[:])
                # v: token-major [128 tok, FG], scattered into [1|v_h] slots
                for t in range(NT):
                    ps = ps_qkv.tile([P, 512], F32, tag="qkv", name="qkv")
                    for k in range(KC):
                        nc.tensor.matmul(
                            ps[:],
                            lhsT=xt[k][:, t * P:(t + 1) * P],
                            rhs=wv[k][:],
                            start=(k == 0),
                            stop=(k == KC - 1),
                        )
                    vv = vag[t].rearrange("p (h c) -> p h c", c=D + 1)
                    nc.any.memset(vv[:, :, D:D + 1], 1.0)
                    nc.any.tensor_copy(
                        vv[:, :, 0:D],
                        ps.rearrange("p (h c) -> p h c", c=D),
                    )
                    if v_bias:
                        nc.vector.tensor_add(vag[t][:], vag[t][:], bv_t[:])

            # ---- phase 2: banded attention -------------------------------
            # PV uses E^T blocks as the stationary operand so the output is
            # A natural [tq, 65] with the softmax denominator in column 64 —
            # per-partition, so normalization is reciprocal + tensor_scalar.
            with tc.tile_pool(name="ps_s", bufs=3, space="PSUM") as ps_s, \
                 tc.tile_pool(name="ps_a", bufs=5, space="PSUM") as ps_a:
                for h in range(HG):
                    hp, ho = h // 2, (h % 2) * D
                    q_ap = qkT[hp]
                    k_ap = qkT[4 + hp]
                    psA = {}
                    for i in range(NT):
                        w = min(W3, (NT - i) * P)
                        ps = ps_s.tile([P, W3], F32, tag="s", name="s")
                        nc.tensor.matmul(
                            ps[:, :w],
                            lhsT=k_ap[ho:ho + D, i * P:(i + 1) * P],
                            rhs=q_ap[ho:ho + D, i * P:i * P + w],
                            start=True, stop=True,
                        )
                        e_t = e_pool.tile([P, W3], BF16, tag="e", name="e")
                        nc.scalar.activation(
                            e_t[:, :w], ps[:, :w],
                            mybir.ActivationFunctionType.Exp,
                            scale=0.125,
                        )
                        nc.vector.tensor_mul(e_t[:, :w], e_t[:, :w], mask_t[:, :w])
                        for dj in range(3):
                            j = i + dj
                            if j >= NT:
                                continue
                            first = (i == max(0, j - 2))
                            last = (i == j)
                            if first:
                                psA[j] = ps_a.tile([P, D + 1], F32, tag="psA", name="psA")
                            nc.tensor.matmul(
                                psA[j][:],
                                lhsT=e_t[:, dj * P:(dj + 1) * P],
                                rhs=vag[i][:, h * (D + 1):(h + 1) * (D + 1)],
                                start=first, stop=last,
                            )
                            if last:
                                pa = psA.pop(j)
                                rs = stage_p.tile([P, 1], F32, tag="rs", name="rs")
                                nc.vector.reciprocal(rs[:], pa[:, D:D + 1])
                                nc.vector.tensor_scalar(
                                    a_nat[j][:, h * D:(h + 1) * D],
                                    pa[:, 0:D],
                                    rs[:],
                                    None,
                                    mybir.AluOpType.mult,
                                )
                                if h % 2 == 1:
                                    # head pair (h-1, h) done for this j:
                                    # transpose the finished 128-col block
                                    k2 = h // 2
                                    nc.sync.dma_start_transpose(
                                        aTb[k2][:, j * P:(j + 1) * P],
                                        a_nat[j][:, k2 * P:(k2 + 1) * P],
                                    )
                    assert not psA

            # ---- phase 3: output projection ------------------------------
            with tc.tile_pool(name="ps_y", bufs=3, space="PSUM") as ps_y:
                for j in range(NT):
                    yt = y_pool.tile([P, C], F32, tag="y", name="y")
                    for n in range(C // 512):
                        ps = ps_y.tile([P, 512], F32, tag="py", name="py")
                        for k2 in range(FG // P):
                            nc.tensor.matmul(
                                ps[:],
                                lhsT=aTb[k2][:, j * P:(j + 1) * P],
                                rhs=wp[k2][:, n * 512:(n + 1) * 512],
                                start=(k2 == 0),
                                stop=(k2 == FG // P - 1),
                            )
                        nc.vector.tensor_copy(yt[:, n * 512:(n + 1) * 512], ps[:])
                    nc.sync.dma_start(y_d[j * P:(j + 1) * P, :], yt[:])

    nc.finalize()
    return nc


def _band_mask_strip() -> np.ndarray:
    tk = np.arange(P)[:, None]
    tq = np.arange(W3)[None, :]
    return (((tq - tk) >= 0) & ((tq - tk) < CTX)).astype(ml_dtypes.bfloat16)


def kernel(x, W_attn, b_attn, W_proj, b_proj):
    global LAST_RESULTS
    x = np.asarray(x, dtype=np.float32)
    W_attn = np.asarray(W_attn, dtype=np.float32)
    b_attn = np.asarray(b_attn, dtype=np.float32)
    W_proj = np.asarray(W_proj, dtype=np.float32)
    b_proj = np.asarray(b_proj, dtype=np.float32)

    qk_bias = bool(np.any(b_attn[:2 * C]))
    v_bias = bool(np.any(b_attn[2 * C:]))

    key = (qk_bias, v_bias)
    if key not in _BUILD_CACHE:
        _BUILD_CACHE[key] = _build_nc(qk_bias, v_bias)
    nc = _BUILD_CACHE[key]

    mstrip = _band_mask_strip()
    in_maps = []
    for c in range(8):
        b, g = c // 2, c % 2
        fsl = slice(FG * g, FG * (g + 1))
        im = {
            "xt": np.ascontiguousarray(x[b].T).astype(ml_dtypes.bfloat16),
            "wqk": np.ascontiguousarray(
                np.concatenate(
                    [W_attn[:, fsl], W_attn[:, C + FG * g:C + FG * (g + 1)]],
                    axis=1,
                )
            ).astype(ml_dtypes.bfloat16),
            "wv": np.ascontiguousarray(
                W_attn[:, 2 * C + FG * g:2 * C + FG * (g + 1)]
            ).astype(ml_dtypes.bfloat16),
            "wp": np.ascontiguousarray(W_proj[fsl, :]).astype(ml_dtypes.bfloat16),
            "mstrip": mstrip,
        }
        if qk_bias:
            bq = b_attn[fsl]
            bk = b_attn[C + FG * g:C + FG * (g + 1)]
            im["bqk"] = np.concatenate([bq, bk]).reshape(8, P).astype(np.float32)
        if v_bias:
            bv = b_attn[2 * C + FG * g:2 * C + FG * (g + 1)]
            bvt = np.zeros((HG, D + 1), dtype=np.float32)
            bvt[:, 1:] = bv.reshape(HG, D)
            im["bv"] = np.broadcast_to(
                bvt.reshape(1, HG * (D + 1)), (P, HG * (D + 1))
            ).astype(ml_dtypes.bfloat16)
        in_maps.append(im)

    res = run_bass_kernel_spmd(nc, in_maps, list(range(8)))
    LAST_RESULTS = res

    out = np.empty((B, T, C), dtype=np.float32)
    for b in range(B):
        out[b] = res.results[2 * b]["y"] + res.results[2 * b + 1]["y"] + b_proj
    return out


# revision 18
# speedup vs baseline: 1.5263x; 1.0153x over previous
"""Trainium2 Bass kernel for banded (local-causal) multi-head self-attention.

Problem (hardcoded shapes): x [4, 2048, 1024], W_attn [1024, 3072],
b_attn [3072], W_proj [1024, 1024], b_proj [1024]; 16 heads, head dim 64,
local causal window 256.

Sharding over 8 NeuronCores: data-parallel over the 4 batches x
tensor-parallel over 2 head-groups (8 heads each). Each core computes a
partial projection output [2048, 1024]; the host sums the two head-group
partials per batch and adds b_proj.

Per-core device program (all loops fully unrolled under Tile):
  phase 1: qkv projections.  q^T/k^T produced feature-major [feat, T] so the
           attention contraction (over head dim) has feat on partitions; v is
           produced token-major [T, feat] interleaved with a ones column per
           head ([1 | v_h] stride 65) so the PV matmul also emits the softmax
           denominator.
  phase 2: banded attention per head over T-strips.  For key-block i the
           valid queries are tq in [128i, 128i+384), so one [64x128]^T @
           [64, w] matmul gives the S^T strip, one Exp activation (scale =
           1/sqrt(64)) and one band-mask multiply give the E^T strip, and
           three [128, 65]^T @ [128, 128] matmuls accumulate A^T (rows 1..64)
           and the denominator s (row 0) in PSUM.  s is reciprocal'd,
           broadcast across partitions with a K=1 ones matmul, and multiplied
           into A^T during the PSUM->SBUF drain.
  phase 3: output projection from a^T [512, T] tiles (float32r matmuls).

Matmul dtypes: bf16 for qkv + attention (inputs pre-cast on host), float32r
for the output projection.  All accumulation is fp32 in PSUM.
"""

import os
import numpy as np
import ml_dtypes

import concourse.bass as bass
import concourse.bacc as bacc
import concourse.mybir as mybir
import concourse.tile as tile
from concourse.bass_utils import run_bass_kernel_spmd

B, T, C = 4, 2048, 1024
H, D, CTX = 16, 64, 256
HG = 8                 # heads per core
FG = HG * D            # 512 features per group
P = 128
NT = T // P            # 16 token blocks
KC = C // P            # 8 contraction tiles of C
W3 = 3 * P             # strip width 384

BF16 = mybir.dt.bfloat16
F32 = mybir.dt.float32
F32R = mybir.dt.float32r

# set by the last kernel() call; test harness reads exec_time_ns from here
LAST_RESULTS = None

_BUILD_CACHE = {}


def _build_nc(qk_bias: bool, v_bias: bool) -> bass.Bass:
    nc = bacc.Bacc()

    xt_d = nc.declare_dram_parameter("xt", [C, T], BF16, isOutput=False)
    wqk_d = nc.declare_dram_parameter("wqk", [C, 2 * FG], BF16, isOutput=False)
    wv_d = nc.declare_dram_parameter("wv", [C, FG], BF16, isOutput=False)
    wp_d = nc.declare_dram_parameter("wp", [FG, C], BF16, isOutput=False)
    mstrip_d = nc.declare_dram_parameter("mstrip", [P, W3], BF16, isOutput=False)
    if qk_bias:
        bqk_d = nc.declare_dram_parameter("bqk", [8, P], F32, isOutput=False)
    if v_bias:
        bv_d = nc.declare_dram_parameter("bv", [P, HG * (D + 1)], BF16, isOutput=False)
    y_d = nc.declare_dram_parameter("y", [T, C], F32, isOutput=True)

    with tile.TileContext(nc) as tc:
        with tc.tile_pool(name="const", bufs=1) as const, \
             tc.tile_pool(name="stage", bufs=8) as stage_p, \
             tc.tile_pool(name="epool", bufs=6) as e_pool, \
             tc.tile_pool(name="ypool", bufs=3) as y_pool:

            # ---- resident SBUF tiles -------------------------------------
            xt = [const.tile([P, T], BF16, tag=f"xt{k}", name=f"xt{k}") for k in range(KC)]
            wqk = [const.tile([P, 2 * FG], BF16, tag=f"wqk{k}", name=f"wqk{k}") for k in range(KC)]
            wv = [const.tile([P, FG], BF16, tag=f"wv{k}", name=f"wv{k}") for k in range(KC)]
            wp = [const.tile([P, C], BF16, tag=f"wp{k}", name=f"wp{k}") for k in range(FG // P)]
            qkT = [const.tile([P, T], BF16, tag=f"qkT{f}", name=f"qkT{f}") for f in range(8)]
            vag = [const.tile([P, HG * (D + 1)], BF16, tag=f"vag{t}", name=f"vag{t}") for t in range(NT)]
            a_nat = [const.tile([P, FG], BF16, tag=f"an{t}", name=f"an{t}") for t in range(NT)]
            aTb = [const.tile([P, T], BF16, tag=f"aTb{k}", name=f"aTb{k}") for k in range(FG // P)]
            mask_t = const.tile([P, W3], BF16, tag="mask", name="mask")

            nc.sync.dma_start(mask_t[:], mstrip_d[:])
            xt_r = xt_d.rearrange("(a p) t -> a p t", p=P)
            wqk_r = wqk_d.rearrange("(a p) f -> a p f", p=P)
            wv_r = wv_d.rearrange("(a p) f -> a p f", p=P)
            wp_r = wp_d.rearrange("(a p) f -> a p f", p=P)
            for k in range(KC):
                nc.sync.dma_start(wqk[k][:, 0:FG], wqk_r[k][:, 0:FG])
                nc.sync.dma_start(xt[k][:, 0:T // 2], xt_r[k][:, 0:T // 2])
            for k in range(KC):
                nc.sync.dma_start(wqk[k][:, FG:2 * FG], wqk_r[k][:, FG:2 * FG])
            for k in range(KC):
                nc.sync.dma_start(xt[k][:, T // 2:T], xt_r[k][:, T // 2:T])
                nc.sync.dma_start(wv[k][:], wv_r[k])
            for k in range(FG // P):
                nc.sync.dma_start(wp[k][:], wp_r[k])
            if qk_bias:
                bqk_t = const.tile([P, 8], F32, tag="bqk", name="bqk")
                nc.sync.dma_start(bqk_t[:], bqk_d.rearrange("a p -> p a"))
            if v_bias:
                bv_t = const.tile([P, HG * (D + 1)], BF16, tag="bv", name="bv")
                nc.sync.dma_start(bv_t[:], bv_d[:])

            # ---- phase 1: qkv projections --------------------------------
            with tc.tile_pool(name="ps_qkv", bufs=3, space="PSUM") as ps_qkv:
                # q^T / k^T: feature-major [128 feat, T]
                for ft in range(8):
                    for nt in range(T // 512):
                        ps = ps_qkv.tile([P, 512], F32, tag="qkv", name="qkv")
                        for k in range(KC):
                            nc.tensor.matmul(
                                ps[:],
                                lhsT=wqk[k][:, ft * P:(ft + 1) * P],
                                rhs=xt[k][:, nt * 512:(nt + 1) * 512],
                                start=(k == 0),
                                stop=(k == KC - 1),
                            )
                        dst = qkT[ft][:, nt * 512:(nt + 1) * 512]
                        if qk_bias:
                            nc.scalar.activation(
                                dst, ps[:],
                                mybir.ActivationFunctionType.Copy,
                                bias=bqk_t[:, ft:ft + 1],
                            )
                        else:
                            nc.any.tensor_copy(dst, ps[:])
                # v: token-major [128 tok, FG], scattered into [1|v_h] slots
                for t in range(NT):
                    ps = ps_qkv.tile([P, 512], F32, tag="qkv", name="qkv")
                    for k in range(KC):
                        nc.tensor.matmul(
                            ps[:],
                            lhsT=xt[k][:, t * P:(t + 1) * P],
                            rhs=wv[k][:],
                            start=(k == 0),
                            stop=(k == KC - 1),
                        )
                    vv = vag[t].rearrange("p (h c) -> p h c", c=D + 1)
                    nc.any.memset(vv[:, :, D:D + 1], 1.0)
                    nc.any.tensor_copy(
                        vv[:, :, 0:D],
                        ps.rearrange("p (h c) -> p h c", c=D),
                    )
                    if v_bias:
                        nc.vector.tensor_add(vag[t][:], vag[t][:], bv_t[:])

            # ---- phase 2: banded attention -------------------------------
            # PV uses E^T blocks as the stationary operand so the output is
            # A natural [tq, 65] with the softmax denominator in column 64 —
            # per-partition, so normalization is reciprocal + tensor_scalar.
            # Head pairs are processed together: the odd head's q/k live at
            # partitions 64..127, so the two K=64 S^T matmuls land in
            # disjoint PE row-groups and run concurrently.  The pair's PV
            # accumulators share one PSUM bank ([128, 130] = two 65-col
            # slots).
            with tc.tile_pool(name="ps_s", bufs=4, space="PSUM") as ps_s, \
                 tc.tile_pool(name="ps_a", bufs=4, space="PSUM") as ps_a:
                for hp in range(HG // 2):
                    q_ap = qkT[hp]
                    k_ap = qkT[4 + hp]
                    psA = {}
                    for i in range(NT):
                        w = min(W3, (NT - i) * P)
                        e_pair = []
                        for idx in range(2):
                            ho = idx * D
                            ps = ps_s.tile([P, W3], F32, tag="s", name="s")
                            nc.tensor.matmul(
                                ps[:, :w],
                                lhsT=k_ap[ho:ho + D, i * P:(i + 1) * P],
                                rhs=q_ap[ho:ho + D, i * P:i * P + w],
                                start=True, stop=True,
                            )
                            e_t = e_pool.tile([P, W3], BF16, tag="e", name="e")
                            nc.scalar.activation(
                                e_t[:, :w], ps[:, :w],
                                mybir.ActivationFunctionType.Exp,
                                scale=0.125,
                            )
                            nc.vector.tensor_mul(e_t[:, :w], e_t[:, :w],
                                                 mask_t[:, :w])
                            e_pair.append(e_t)
                        for dj in range(3):
                            j = i + dj
                            if j >= NT:
                                continue
                            first = (i == max(0, j - 2))
                            last = (i == j)
                            if first:
                                psA[j] = ps_a.tile([P, 2 * (D + 1)], F32,
                                                   tag="psA", name="psA")
                            for idx in range(2):
                                h = 2 * hp + idx
                                # start=True clears the whole bank, so only
                                # the pair's first matmul may set it
                                nc.tensor.matmul(
                                    psA[j][:, idx * (D + 1):(idx + 1) * (D + 1)],
                                    lhsT=e_pair[idx][:, dj * P:(dj + 1) * P],
                                    rhs=vag[i][:, h * (D + 1):(h + 1) * (D + 1)],
                                    start=first and idx == 0,
                                    stop=last and idx == 1,
                                    skip_group_check=True,
                                )
                            if last:
                                pa = psA.pop(j)
                                for idx in range(2):
                                    h = 2 * hp + idx
                                    o = idx * (D + 1)
                                    rs = stage_p.tile([P, 1], F32, tag="rs",
                                                      name="rs")
                                    nc.vector.reciprocal(rs[:], pa[:, o + D:o + D + 1])
                                    nc.vector.tensor_scalar(
                                        a_nat[j][:, h * D:(h + 1) * D],
                                        pa[:, o:o + D],
                                        rs[:],
                                        None,
                                        mybir.AluOpType.mult,
                                    )
                                # pair columns done for this j: transpose
                                nc.sync.dma_start_transpose(
                                    aTb[hp][:, j * P:(j + 1) * P],
                                    a_nat[j][:, hp * P:(hp + 1) * P],
                                )
                    assert not psA

            # ---- phase 3: output projection ------------------------------
            with tc.tile_pool(name="ps_y", bufs=3, space="PSUM") as ps_y:
                for j in range(NT):
                    yt = y_pool.tile([P, C], F32, tag="y", name="y")
                    for n in range(C // 512):
                        ps = ps_y.tile([P, 512], F32, tag="py", name="py")
                        for k2 in range(FG // P):
                            nc.tensor.matmul(
                                ps[:],
                                lhsT=aTb[k2][:, j * P:(j + 1) * P],
                                rhs=wp[k2][:, n * 512:(n + 1) * 512],
                                start=(k2 == 0),
                                stop=(k2 == FG // P - 1),
                            )
                        nc.vector.tensor_copy(yt[:, n * 512:(n + 1) * 512], ps[:])
                    nc.sync.dma_start(y_d[j * P:(j + 1) * P, :], yt[:])

    nc.finalize()
    return nc


def _band_mask_strip() -> np.ndarray:
    tk = np.arange(P)[:, None]
    tq = np.arange(W3)[None, :]
    return (((tq - tk) >= 0) & ((tq - tk) < CTX)).astype(ml_dtypes.bfloat16)


def kernel(x, W_attn, b_attn, W_proj, b_proj):
    global LAST_RESULTS
    x = np.asarray(x, dtype=np.float32)
    W_attn = np.asarray(W_attn, dtype=np.float32)
    b_attn = np.asarray(b_attn, dtype=np.float32)
    W_proj = np.asarray(W_proj, dtype=np.float32)
    b_proj = np.asarray(b_proj, dtype=np.float32)

    qk_bias = bool(np.any(b_attn[:2 * C]))
    v_bias = bool(np.any(b_attn[2 * C:]))

    key = (qk_bias, v_bias)
    if key not in _BUILD_CACHE:
        _BUILD_CACHE[key] = _build_nc(qk_bias, v_bias)
    nc = _BUILD_CACHE[key]

    mstrip = _band_mask_strip()
    in_maps = []
    for c in range(8):
        b, g = c // 2, c % 2
        fsl = slice(FG * g, FG * (g + 1))
        im = {
            "xt": np.ascontiguousarray(x[b].T).astype(ml_dtypes.bfloat16),
            "wqk": np.ascontiguousarray(
                np.concatenate(
                    [W_attn[:, fsl], W_attn[:, C + FG * g:C + FG * (g + 1)]],
                    axis=1,
                )
            ).astype(ml_dtypes.bfloat16),
            "wv": np.ascontiguousarray(
                W_attn[:, 2 * C + FG * g:2 * C + FG * (g + 1)]
            ).astype(ml_dtypes.bfloat16),
            "wp": np.ascontiguousarray(W_proj[fsl, :]).astype(ml_dtypes.bfloat16),
            "mstrip": mstrip,
        }
        if qk_bias:
            bq = b_attn[fsl]
            bk = b_attn[C + FG * g:C + FG * (g + 1)]
            im["bqk"] = np.concatenate([bq, bk]).reshape(8, P).astype(np.float32)
        if v_bias:
            bv = b_attn[2 * C + FG * g:2 * C + FG * (g + 1)]
            bvt = np.zeros((HG, D + 1), dtype=np.float32)
            bvt[:, 1:] = bv.reshape(HG, D)
            im["bv"] = np.broadcast_to(
                bvt.reshape(1, HG * (D + 1)), (P, HG * (D + 1))
            ).astype(ml_dtypes.bfloat16)
        in_maps.append(im)

    res = run_bass_kernel_spmd(nc, in_maps, list(range(8)))
    LAST_RESULTS = res

    out = np.empty((B, T, C), dtype=np.float32)
    for b in range(B):
        out[b] = res.results[2 * b]["y"] + res.results[2 * b + 1]["y"] + b_proj
    return out


# revision 22
# speedup vs baseline: 1.6024x; 1.0499x over previous
"""Trainium2 Bass kernel for banded (local-causal) multi-head self-attention.

Problem (hardcoded shapes): x [4, 2048, 1024], W_attn [1024, 3072],
b_attn [3072], W_proj [1024, 1024], b_proj [1024]; 16 heads, head dim 64,
local causal window 256.

Sharding over 8 NeuronCores: data-parallel over the 4 batches x
tensor-parallel over 2 head-groups (8 heads each). Each core computes a
partial projection output [2048, 1024]; the host sums the two head-group
partials per batch and adds b_proj.

Per-core device program (all loops fully unrolled under Tile):
  phase 1: qkv projections.  q^T/k^T produced feature-major [feat, T] so the
           attention contraction (over head dim) has feat on partitions; v is
           produced token-major [T, feat] interleaved with a ones column per
           head ([1 | v_h] stride 65) so the PV matmul also emits the softmax
           denominator.
  phase 2: banded attention per head over T-strips.  For key-block i the
           valid queries are tq in [128i, 128i+384), so one [64x128]^T @
           [64, w] matmul gives the S^T strip, one Exp activation (scale =
           1/sqrt(64)) and one band-mask multiply give the E^T strip, and
           three [128, 65]^T @ [128, 128] matmuls accumulate A^T (rows 1..64)
           and the denominator s (row 0) in PSUM.  s is reciprocal'd,
           broadcast across partitions with a K=1 ones matmul, and multiplied
           into A^T during the PSUM->SBUF drain.
  phase 3: output projection from a^T [512, T] tiles (float32r matmuls).

Matmul dtypes: bf16 for qkv + attention (inputs pre-cast on host), float32r
for the output projection.  All accumulation is fp32 in PSUM.
"""

import os
import numpy as np
import ml_dtypes

import concourse.bass as bass
import concourse.bacc as bacc
import concourse.mybir as mybir
import concourse.tile as tile
from concourse.bass_utils import run_bass_kernel_spmd

B, T, C = 4, 2048, 1024
H, D, CTX = 16, 64, 256
HG = 8                 # heads per core
FG = HG * D            # 512 features per group
P = 128
NT = T // P            # 16 token blocks
KC = C // P            # 8 contraction tiles of C
W3 = 3 * P             # strip width 384

BF16 = mybir.dt.bfloat16
F32 = mybir.dt.float32
F32R = mybir.dt.float32r

# set by the last kernel() call; test harness reads exec_time_ns from here
LAST_RESULTS = None

_BUILD_CACHE = {}


def _build_nc(qk_bias: bool, v_bias: bool) -> bass.Bass:
    nc = bacc.Bacc()

    xt_d = nc.declare_dram_parameter("xt", [C, T], BF16, isOutput=False)
    wqk_d = nc.declare_dram_parameter("wqk", [C, 2 * FG], BF16, isOutput=False)
    wv_d = nc.declare_dram_parameter("wv", [C, FG], BF16, isOutput=False)
    wp_d = nc.declare_dram_parameter("wp", [FG, C], BF16, isOutput=False)
    mstrip_d = nc.declare_dram_parameter("mstrip", [P, W3], BF16, isOutput=False)
    if qk_bias:
        bqk_d = nc.declare_dram_parameter("bqk", [8, P], F32, isOutput=False)
    if v_bias:
        bv_d = nc.declare_dram_parameter("bv", [P, HG * (D + 1)], BF16, isOutput=False)
    y_d = nc.declare_dram_parameter("y", [T, C], F32, isOutput=True)

    with tile.TileContext(nc) as tc:
        with tc.tile_pool(name="const", bufs=1) as const, \
             tc.tile_pool(name="stage", bufs=8) as stage_p, \
             tc.tile_pool(name="epool", bufs=6) as e_pool, \
             tc.tile_pool(name="ypool", bufs=3) as y_pool:

            # ---- resident SBUF tiles -------------------------------------
            xt = [const.tile([P, T], BF16, tag=f"xt{k}", name=f"xt{k}") for k in range(KC)]
            wqk = [const.tile([P, 2 * FG], BF16, tag=f"wqk{k}", name=f"wqk{k}") for k in range(KC)]
            wv = [const.tile([P, FG], BF16, tag=f"wv{k}", name=f"wv{k}") for k in range(KC)]
            wp = [const.tile([P, C], BF16, tag=f"wp{k}", name=f"wp{k}") for k in range(FG // P)]
            qkT = [const.tile([P, T], BF16, tag=f"qkT{f}", name=f"qkT{f}") for f in range(8)]
            vag = [const.tile([P, HG * (D + 1)], BF16, tag=f"vag{t}", name=f"vag{t}") for t in range(NT)]
            a_nat = [const.tile([P, FG], BF16, tag=f"an{t}", name=f"an{t}") for t in range(NT)]
            aTb = [const.tile([P, T], BF16, tag=f"aTb{k}", name=f"aTb{k}") for k in range(FG // P)]
            mask_t = const.tile([P, W3], BF16, tag="mask", name="mask")

            nc.sync.dma_start(mask_t[:], mstrip_d[:])
            xt_r = xt_d.rearrange("(a p) t -> a p t", p=P)
            wqk_r = wqk_d.rearrange("(a p) f -> a p f", p=P)
            wv_r = wv_d.rearrange("(a p) f -> a p f", p=P)
            wp_r = wp_d.rearrange("(a p) f -> a p f", p=P)
            for k in range(KC):
                nc.sync.dma_start(wqk[k][:, 0:FG], wqk_r[k][:, 0:FG])
                nc.sync.dma_start(xt[k][:, 0:T // 2], xt_r[k][:, 0:T // 2])
            for k in range(KC):
                nc.sync.dma_start(wqk[k][:, FG:2 * FG], wqk_r[k][:, FG:2 * FG])
            for k in range(KC):
                nc.sync.dma_start(xt[k][:, T // 2:T], xt_r[k][:, T // 2:T])
                nc.sync.dma_start(wv[k][:], wv_r[k])
            for k in range(FG // P):
                nc.sync.dma_start(wp[k][:], wp_r[k])
            if qk_bias:
                bqk_t = const.tile([P, 8], F32, tag="bqk", name="bqk")
                nc.sync.dma_start(bqk_t[:], bqk_d.rearrange("a p -> p a"))
            if v_bias:
                bv_t = const.tile([P, HG * (D + 1)], BF16, tag="bv", name="bv")
                nc.sync.dma_start(bv_t[:], bv_d[:])

            # ---- phases 1+2 fused: qkv projections + banded attention ----
            # The attention matmuls are small (N<=384) and alone don't
            # register as "busy" to the PE clock-gate (HAM) — the whole
            # attention phase then runs at 1.2 GHz.  Interleaving each head
            # pair's attention with the NEXT pair's dense q/k projection
            # matmuls (N=512) keeps the PE at 2.4 GHz and fills stalls.
            with tc.tile_pool(name="ps_qkv", bufs=2, space="PSUM") as ps_qkv, \
                 tc.tile_pool(name="ps_s", bufs=2, space="PSUM") as ps_s, \
                 tc.tile_pool(name="ps_a", bufs=4, space="PSUM") as ps_a:

                def emit_qk_tile(hp, idx):
                    # one [128, 512] output tile of the pair's q/k projection
                    ft = hp if idx < 4 else 4 + hp
                    nt = idx % 4
                    ps = ps_qkv.tile([P, 512], F32, tag="qkv", name="qkv")
                    for k in range(KC):
                        nc.tensor.matmul(
                            ps[:],
                            lhsT=wqk[k][:, ft * P:(ft + 1) * P],
                            rhs=xt[k][:, nt * 512:(nt + 1) * 512],
                            start=(k == 0),
                            stop=(k == KC - 1),
                        )
                    dst = qkT[ft][:, nt * 512:(nt + 1) * 512]
                    if qk_bias:
                        nc.scalar.activation(
                            dst, ps[:],
                            mybir.ActivationFunctionType.Copy,
                            bias=bqk_t[:, ft:ft + 1],
                        )
                    else:
                        nc.any.tensor_copy(dst, ps[:])

                def emit_qk_pair(hp):
                    for idx in range(8):
                        emit_qk_tile(hp, idx)

                # v first: token-major [128 tok, FG], into [v_h|1] slots
                for t in range(NT):
                    ps = ps_qkv.tile([P, 512], F32, tag="qkv", name="qkv")
                    for k in range(KC):
                        nc.tensor.matmul(
                            ps[:],
                            lhsT=xt[k][:, t * P:(t + 1) * P],
                            rhs=wv[k][:],
                            start=(k == 0),
                            stop=(k == KC - 1),
                        )
                    vv = vag[t].rearrange("p (h c) -> p h c", c=D + 1)
                    nc.any.memset(vv[:, :, D:D + 1], 1.0)
                    nc.any.tensor_copy(
                        vv[:, :, 0:D],
                        ps.rearrange("p (h c) -> p h c", c=D),
                    )
                    if v_bias:
                        nc.vector.tensor_add(vag[t][:], vag[t][:], bv_t[:])
                emit_qk_pair(0)

                # attention per head pair; the odd head's q/k live at
                # partitions 64..127, so the two K=64 S^T matmuls land in
                # disjoint PE row-groups and run concurrently.  The pair's
                # PV accumulators share one PSUM bank ([128, 130]).
                for hp in range(HG // 2):
                    q_ap = qkT[hp]
                    k_ap = qkT[4 + hp]
                    psA = {}
                    for i in range(NT):
                        # spread the next pair's q/k projection through this
                        # pair's attention so the PE always sees dense work
                        if hp + 1 < HG // 2 and i % 2 == 0:
                            emit_qk_tile(hp + 1, i // 2)
                        w = min(W3, (NT - i) * P)
                        e_pair = []
                        for idx in range(2):
                            ho = idx * D
                            ps = ps_s.tile([P, W3], F32, tag="s", name="s")
                            nc.tensor.matmul(
                                ps[:, :w],
                                lhsT=k_ap[ho:ho + D, i * P:(i + 1) * P],
                                rhs=q_ap[ho:ho + D, i * P:i * P + w],
                                start=True, stop=True,
                            )
                            e_t = e_pool.tile([P, W3], BF16, tag="e", name="e")
                            nc.scalar.activation(
                                e_t[:, :w], ps[:, :w],
                                mybir.ActivationFunctionType.Exp,
                                scale=0.125,
                            )
                            nc.vector.tensor_mul(e_t[:, :w], e_t[:, :w],
                                                 mask_t[:, :w])
                            e_pair.append(e_t)
                        for dj in range(3):
                            j = i + dj
                            if j >= NT:
                                continue
                            first = (i == max(0, j - 2))
                            last = (i == j)
                            if first:
                                psA[j] = ps_a.tile([P, 2 * (D + 1)], F32,
                                                   tag="psA", name="psA")
                            for idx in range(2):
                                h = 2 * hp + idx
                                # start=True clears the whole bank, so only
                                # the pair's first matmul may set it
                                nc.tensor.matmul(
                                    psA[j][:, idx * (D + 1):(idx + 1) * (D + 1)],
                                    lhsT=e_pair[idx][:, dj * P:(dj + 1) * P],
                                    rhs=vag[i][:, h * (D + 1):(h + 1) * (D + 1)],
                                    start=first and idx == 0,
                                    stop=last and idx == 1,
                                    skip_group_check=True,
                                )
                            if last:
                                pa = psA.pop(j)
                                for idx in range(2):
                                    h = 2 * hp + idx
                                    o = idx * (D + 1)
                                    rs = stage_p.tile([P, 1], F32, tag="rs",
                                                      name="rs")
                                    nc.vector.reciprocal(rs[:], pa[:, o + D:o + D + 1])
                                    nc.vector.tensor_scalar(
                                        a_nat[j][:, h * D:(h + 1) * D],
                                        pa[:, o:o + D],
                                        rs[:],
                                        None,
                                        mybir.AluOpType.mult,
                                    )
                                # pair columns done for this j: transpose
                                nc.sync.dma_start_transpose(
                                    aTb[hp][:, j * P:(j + 1) * P],
                                    a_nat[j][:, hp * P:(hp + 1) * P],
                                )
                    assert not psA

            # ---- phase 3: output projection ------------------------------
            with tc.tile_pool(name="ps_y", bufs=3, space="PSUM") as ps_y:
                for j in range(NT):
                    yt = y_pool.tile([P, C], F32, tag="y", name="y")
                    for n in range(C // 512):
                        ps = ps_y.tile([P, 512], F32, tag="py", name="py")
                        for k2 in range(FG // P):
                            nc.tensor.matmul(
                                ps[:],
                                lhsT=aTb[k2][:, j * P:(j + 1) * P],
                                rhs=wp[k2][:, n * 512:(n + 1) * 512],
                                start=(k2 == 0),
                                stop=(k2 == FG // P - 1),
                            )
                        nc.vector.tensor_copy(yt[:, n * 512:(n + 1) * 512], ps[:])
                    # gpsimd DGE: keeps y writes off the sync queue, which
                    # blocks on the interleaved aTb transposes
                    nc.gpsimd.dma_start(y_d[j * P:(j + 1) * P, :], yt[:])

    nc.finalize()
    return nc


def _band_mask_strip() -> np.ndarray:
    tk = np.arange(P)[:, None]
    tq = np.arange(W3)[None, :]
    return (((tq - tk) >= 0) & ((tq - tk) < CTX)).astype(ml_dtypes.bfloat16)


def kernel(x, W_attn, b_attn, W_proj, b_proj):
    global LAST_RESULTS
    x = np.asarray(x, dtype=np.float32)
    W_attn = np.asarray(W_attn, dtype=np.float32)
    b_attn = np.asarray(b_attn, dtype=np.float32)
    W_proj = np.asarray(W_proj, dtype=np.float32)
    b_proj = np.asarray(b_proj, dtype=np.float32)

    qk_bias = bool(np.any(b_attn[:2 * C]))
    v_bias = bool(np.any(b_attn[2 * C:]))

    key = (qk_bias, v_bias)
    if key not in _BUILD_CACHE:
        _BUILD_CACHE[key] = _build_nc(qk_bias, v_bias)
    nc = _BUILD_CACHE[key]

    mstrip = _band_mask_strip()
    in_maps = []
    for c in range(8):
        b, g = c // 2, c % 2
        fsl = slice(FG * g, FG * (g + 1))
        im = {
            "xt": np.ascontiguousarray(x[b].T).astype(ml_dtypes.bfloat16),
            "wqk": np.ascontiguousarray(
                np.concatenate(
                    [W_attn[:, fsl], W_attn[:, C + FG * g:C + FG * (g + 1)]],
                    axis=1,
                )
            ).astype(ml_dtypes.bfloat16),
            "wv": np.ascontiguousarray(
                W_attn[:, 2 * C + FG * g:2 * C + FG * (g + 1)]
            ).astype(ml_dtypes.bfloat16),
            "wp": np.ascontiguousarray(W_proj[fsl, :]).astype(ml_dtypes.bfloat16),
            "mstrip": mstrip,
        }
        if qk_bias:
            bq = b_attn[fsl]
            bk = b_attn[C + FG * g:C + FG * (g + 1)]
            im["bqk"] = np.concatenate([bq, bk]).reshape(8, P).astype(np.float32)
        if v_bias:
            bv = b_attn[2 * C + FG * g:2 * C + FG * (g + 1)]
            bvt = np.zeros((HG, D + 1), dtype=np.float32)
            bvt[:, 1:] = bv.reshape(HG, D)
            im["bv"] = np.broadcast_to(
                bvt.reshape(1, HG * (D + 1)), (P, HG * (D + 1))
            ).astype(ml_dtypes.bfloat16)
        in_maps.append(im)

    res = run_bass_kernel_spmd(nc, in_maps, list(range(8)))
    LAST_RESULTS = res

    out = np.empty((B, T, C), dtype=np.float32)
    for b in range(B):
        out[b] = res.results[2 * b]["y"] + res.results[2 * b + 1]["y"] + b_proj
    return out
